# revision 15
# baseline (speedup 1.0000x reference)
"""AutoRec forward kernel for Trainium2, 8-core SPMD.

Math (see reference):
    agg = segment_sum(r[:,None] * v[cols], rows, m)     # sparse (m,n) @ v
    h   = sigmoid(agg + mu)                             # (M, D)
    s   = sum(h[i] * w[j])                              # global scalar over E pairs
    out = s + b[j]                                      # (E,)

Device strategy (per core, users sharded):
  Each core owns RPC = 6272 rows (users). Both heavy stages are instances of
  one primitive: "gather rows from a replicated table, weight them, and
  segment-sum into a local per-row accumulator":
    phase 1: table=v (bf16), weights=r,     rows=ij[0], cols=ij[1] -> aggT
    phase 2: table=w (f32),  weights=1.0,   rows=i,     cols=j     -> aT
          (sum_e h[i_e] * w[j_e] = sum_u h[u] . A[u],  A[u] = sum_{i_e=u} w[j_e])
  The segment-sum runs on the tensor engine: for each chunk of 128 edges the
  gathered rows form the stationary operand [128e, 128d]; a one-hot matrix
  P[e, wrow] = weight_e * (local_row_e == wrow) built on DVE is the moving
  operand; psum accumulates aggT[d, wrow] over a 64-row window. Edges are
  pre-sorted by (table-half, window) on the host so windows are contiguous,
  and the static schedule (max chunk count per group across cores) is shared
  by all cores so one SPMD program serves all 8.
  Tables are split in two 25000-row halves because dma_gather indices are
  int16. Finally h = sigmoid(aggT + mu) in one ACT op and
  s_part = sum(hT * aT) reduced on DVE; the host sums the 8 partials and
  broadcasts s + b[j] (a trivial O(E) numpy gather).
"""

import math
from dataclasses import dataclass, field

import ml_dtypes
import numpy as np

# ---------------------------------------------------------------- config

CHUNK = 128  # edges per matmul (contraction = partition dim)
IDX_WRAP = 16  # dma_gather index wrap


@dataclass
class Cfg:
    M: int = 50000          # users (rows of spmm)
    dma_scratch: int = 65536  # SWDGE descriptor carveout (bytes)
    N: int = 50000          # items (table rows)
    D: int = 128            # feature dim (must be 128)
    ncores: int = 8
    rpc: int = 6272         # rows per core (multiple of window)
    window: int = 128       # psum row-window
    half: int = 25000       # table split (int16 index limit)
    call_chunks: int = 8   # chunks per dma_gather call (ring = dma_scratch/16 descs/queue)
    p1dt: str = "f16"       # value dtype of phase-1 gathers / one-hot
    p2dt: str = "f16"       # value dtype of phase-2 gathers / one-hot
    ttb: int = 512          # block size of the final fused mul-reduce
    queues: int = 4         # SWDGE queues to round-robin gather calls over
    host_p: bool = False    # precompute one-hot P on host, stream via HWDGE

    @property
    def nwin(self):
        return self.rpc // self.window

    def __post_init__(self):
        assert self.rpc % self.window == 0
        assert self.rpc * self.ncores >= self.M
        assert self.N <= 2 * self.half and self.half <= 32767
        assert self.D == 128


FULL = Cfg()

# ---------------------------------------------------------------- host plan


@dataclass
class PhasePlan:
    groups: list          # [(hf, win, n_chunks)] in stream order (hf-major)
    calls: list           # [(hf, chunk_start, n_chunks)]
    total_chunks: int
    # per-core packed arrays
    idx_dram: list        # [ncores] int16 [128, total_chunks*8]
    wgt_dram: list        # [ncores] f16 [128, total_chunks*128/128] per-edge weights
    rl_dram: list         # [ncores] f16 [128, total_chunks] per-edge local row (-1 pad)


def _wrap_idxs(ii: np.ndarray) -> np.ndarray:
    """[n] -> [128, n/16] wrapped (t -> (t%16, t//16)), replicated x8."""
    n = len(ii)
    a = ii.reshape(n // IDX_WRAP, IDX_WRAP).T
    return np.tile(a, (8, 1))


def plan_phase(cfg: Cfg, rows, cols, wgts) -> PhasePlan:
    rows = np.asarray(rows, np.int64)
    cols = np.asarray(cols, np.int64)
    wgts = np.asarray(wgts, np.float32)
    nwin, ncores, Wd = cfg.nwin, cfg.ncores, cfg.window

    core = rows // cfg.rpc
    local = rows - core * cfg.rpc
    win = local // Wd
    rl = (local - win * Wd).astype(np.float32)
    hf = (cols >= cfg.half).astype(np.int64)
    idx16 = (cols - hf * cfg.half).astype(np.int16)

    key = (core * 2 + hf) * nwin + win
    counts = np.bincount(key, minlength=ncores * 2 * nwin).reshape(ncores, 2, nwin)
    nch = -(-counts.max(axis=0) // CHUNK)  # [2, nwin] ceil
    groups = []
    gbase = np.zeros((2, nwin), np.int64)
    acc = 0
    for h in range(2):
        for w in range(nwin):
            n = int(nch[h, w])
            if n == 0:
                continue
            groups.append((h, w, n))
            gbase[h, w] = acc
            acc += n
    total_chunks = acc

    # gather calls: split each half's chunk-range into spans of call_chunks
    calls = []
    cur = 0
    for h in range(2):
        nh = int(nch[h][counts.max(axis=0)[h] > 0].sum()) if nwin else 0
        # recompute exactly: chunks of half h
        nh = sum(n for (hh, _, n) in groups if hh == h)
        off = cur
        while off < cur + nh:
            n = min(cfg.call_chunks, cur + nh - off)
            calls.append((h, off, n))
            off += n
        cur += nh
    assert cur == total_chunks

    idx_l, wgt_l, rl_l = [], [], []
    for c in range(ncores):
        mask = core == c
        eh, ew = hf[mask], win[mask]
        erl, ei, ewgt = rl[mask], idx16[mask], wgts[mask]
        order = np.lexsort((ew, eh))
        eh, ew, erl, ei, ewgt = (a[order] for a in (eh, ew, erl, ei, ewgt))
        gid = eh * nwin + ew
        # rank within each (hf,win) run of the sorted list
        if len(gid):
            first = np.r_[True, gid[1:] != gid[:-1]]
            run_start = np.maximum.accumulate(np.where(first, np.arange(len(gid)), 0))
            rank = np.arange(len(gid)) - run_start
        else:
            rank = np.zeros(0, np.int64)
        pos = gbase[eh, ew] * CHUNK + rank
        idx_full = np.zeros(total_chunks * CHUNK, np.int16)
        wgt_full = np.zeros(total_chunks * CHUNK, np.float32)
        # pad slots: rl=-1 so the device one-hot (iota==rl) is all-zero
        rl_full = np.full(total_chunks * CHUNK, -1.0, np.float32)
        idx_full[pos] = ei
        wgt_full[pos] = ewgt
        rl_full[pos] = erl

        # wrap idx per call
        parts = []
        for (_h, c0, n) in calls:
            parts.append(_wrap_idxs(idx_full[c0 * CHUNK:(c0 + n) * CHUNK]))
        idx_l.append(np.concatenate(parts, axis=1))
        wgt_l.append(wgt_full.reshape(-1, CHUNK).T.astype(np.float16))
        rl_l.append(rl_full.reshape(-1, CHUNK).T.astype(np.float16))

    return PhasePlan(groups, calls, total_chunks, idx_l, wgt_l, rl_l)


# ---------------------------------------------------------------- device build


def build_program(cfg: Cfg, ph1: PhasePlan, ph2: PhasePlan):
    import concourse.bacc as bacc
    import concourse.bass as bass
    import concourse.mybir as mybir
    import concourse.tile as tile

    f32 = mybir.dt.float32
    f16 = mybir.dt.float16
    i16 = mybir.dt.int16
    DTMAP = {"f32": f32, "bf16": mybir.dt.bfloat16, "f16": mybir.dt.float16}
    p1dt, p2dt = DTMAP[cfg.p1dt], DTMAP[cfg.p2dt]
    P, Wd, RPC = 128, cfg.window, cfg.rpc
    n_hi = cfg.N - cfg.half

    nc = bacc.Bacc("TRN2", target_bir_lowering=False, debug=False,
                   dynamic_dma_scratch_size=cfg.dma_scratch,
                   num_swdge_queues=cfg.queues)

    v_lo = nc.dram_tensor("v_lo", [cfg.half, cfg.D], p1dt, kind="ExternalInput")
    v_hi = nc.dram_tensor("v_hi", [n_hi, cfg.D], p1dt, kind="ExternalInput")
    w_lo = nc.dram_tensor("w_lo", [cfg.half, cfg.D], p2dt, kind="ExternalInput")
    w_hi = nc.dram_tensor("w_hi", [n_hi, cfg.D], p2dt, kind="ExternalInput")
    mu_c = nc.dram_tensor("mu_col", [P, 1], f32, kind="ExternalInput")

    idx1 = nc.dram_tensor("idx1", [P, ph1.total_chunks * 8], i16,
                          kind="ExternalInput")
    idx2 = nc.dram_tensor("idx2", [P, ph2.total_chunks * 8], i16,
                          kind="ExternalInput")
    wgt1 = nc.dram_tensor("wgt1", [P, ph1.total_chunks], f16,
                          kind="ExternalInput")
    rl1 = nc.dram_tensor("rl1", [P, ph1.total_chunks], f16,
                         kind="ExternalInput")
    rl2 = nc.dram_tensor("rl2", [P, ph2.total_chunks], f16,
                         kind="ExternalInput")
    s_out = nc.dram_tensor("s_out", [P, 1], f32, kind="ExternalOutput")

    with tile.TileContext(nc) as tc:
        with (
            tc.tile_pool(name="const", bufs=1) as cpool,
            tc.tile_pool(name="idxp", bufs=8) as ipool,
            tc.tile_pool(name="g1", bufs=6) as g1pool,
            tc.tile_pool(name="g2", bufs=6) as g2pool,
            tc.tile_pool(name="pp", bufs=6) as ppool,
            tc.tile_pool(name="ev", bufs=4) as evpool,
            tc.tile_pool(name="psum", bufs=8, space="PSUM") as pspool,
        ):
            # constants
            mu_t = cpool.tile([P, 1], f32, tag="mu")
            nc.sync.dma_start(mu_t[:], mu_c[:])
            iota1 = cpool.tile([P, Wd], p1dt, tag="iota1")
            nc.gpsimd.iota(iota1[:], pattern=[[1, Wd]], base=0,
                           channel_multiplier=0,
                           allow_small_or_imprecise_dtypes=True)
            iota2 = iota1

            acc1 = cpool.tile([P, RPC], f32, tag="acc1")
            acc2 = cpool.tile([P, RPC], f32, tag="acc2")
            nc.vector.memset(acc1[:], 0.0)
            nc.vector.memset(acc2[:], 0.0)

            wg1_t = cpool.tile([P, ph1.total_chunks], f16, tag="wg1")
            rl1_t = cpool.tile([P, ph1.total_chunks], f16, tag="rl1")
            rl2_t = cpool.tile([P, ph2.total_chunks], f16, tag="rl2")
            nc.sync.dma_start(wg1_t[:], wgt1[:])
            nc.sync.dma_start(rl1_t[:], rl1[:])
            nc.sync.dma_start(rl2_t[:], rl2[:])

            qcount = [0]

            def run_phase(pl: PhasePlan, tabs, idx_dram, wg_t, rl_t,
                          acc, gpool, pdt, io_t):
                # group bookkeeping: map chunk id -> (group, first?, last?)
                chunk_group = {}
                for g, (h, w, n) in enumerate(pl.groups):
                    base = sum(nn for (_, _, nn) in pl.groups[:g])
                    for k in range(n):
                        chunk_group[base + k] = (g, w, k == 0, k == n - 1)
                # consume calls in order, carrying the open psum group
                open_ps = None
                for (h, c0, n) in pl.calls:
                    it = ipool.tile([P, cfg.call_chunks * 8], i16, tag="idx")
                    nc.sync.dma_start(it[:, : n * 8],
                                      idx_dram[:, c0 * 8:(c0 + n) * 8])
                    gt = gpool.tile([P, cfg.call_chunks, cfg.D], pdt, tag="g")
                    nidx = n * CHUNK
                    nc.gpsimd.dma_gather(
                        gt[:, :n, :], tabs[h][:], it[:, : n * 8],
                        num_idxs=nidx, num_idxs_reg=nidx, elem_size=cfg.D,
                        queue_num=qcount[0] % cfg.queues,
                    )
                    qcount[0] += 1
                    # one-hot for the whole call in two (or one) DVE ops:
                    # P[p, k, x] = wgt[p, c0+k] * (iota[x] == rl[p, c0+k])
                    p_t = ppool.tile([P, cfg.call_chunks, Wd], pdt, tag="p")
                    nc.vector.tensor_tensor(
                        out=p_t[:, :n, :],
                        in0=io_t[:, None, :].broadcast_to([P, n, Wd]),
                        in1=rl_t[:, c0:c0 + n, None].broadcast_to([P, n, Wd]),
                        op=mybir.AluOpType.is_equal)
                    if wg_t is not None:
                        nc.vector.tensor_tensor(
                            out=p_t[:, :n, :], in0=p_t[:, :n, :],
                            in1=wg_t[:, c0:c0 + n, None].broadcast_to(
                                [P, n, Wd]),
                            op=mybir.AluOpType.mult)
                    for k in range(n):
                        cid = c0 + k
                        g, w, first, last = chunk_group[cid]
                        if first:
                            open_ps = pspool.tile([P, Wd], mybir.dt.float32,
                                                  tag="ps")
                        nc.tensor.matmul(open_ps[:], gt[:, k, :],
                                         p_t[:, k, :], start=first, stop=last)
                        if last:
                            sl = acc[:, w * Wd:(w + 1) * Wd]
                            nc.vector.tensor_tensor(
                                out=sl, in0=sl, in1=open_ps[:],
                                op=mybir.AluOpType.add)

            run_phase(ph1, (v_lo, v_hi), idx1, wg1_t, rl1_t, acc1,
                      g1pool, p1dt, iota1)
            run_phase(ph2, (w_lo, w_hi), idx2, None, rl2_t, acc2,
                      g2pool, p2dt, iota2)

            # h = sigmoid(aggT + mu)  (in place on acc1)
            nc.scalar.activation(acc1[:], acc1[:],
                                 mybir.ActivationFunctionType.Sigmoid,
                                 bias=mu_t[:, 0:1], scale=1.0)

            # s_part[p] = sum_d sum_u h[p,u]*A[p,u]  blockwise fused mul+reduce
            nblk = math.ceil(RPC / cfg.ttb)
            s_cols = cpool.tile([P, nblk], f32, tag="scols")
            for b in range(nblk):
                lo = b * cfg.ttb
                hi = min(RPC, lo + cfg.ttb)
                tmp = evpool.tile([P, cfg.ttb], f32, tag="tmp")
                nc.vector.tensor_tensor(
                    out=tmp[:, : hi - lo],
                    in0=acc1[:, lo:hi], in1=acc2[:, lo:hi],
                    op=mybir.AluOpType.mult)
                nc.vector.tensor_reduce(
                    s_cols[:, b:b + 1], tmp[:, : hi - lo],
                    axis=mybir.AxisListType.X, op=mybir.AluOpType.add)
            s_t = cpool.tile([P, 1], f32, tag="sfin")
            nc.vector.tensor_reduce(s_t[:], s_cols[:], axis=mybir.AxisListType.X,
                                    op=mybir.AluOpType.add)
            nc.sync.dma_start(s_out[:], s_t[:])

    # Align SWDGE queue assignment with Tile's DMASW lane assignment (which
    # follows scheduled order): queue = lane % queues. A mismatch trips
    # "sem locked to SWDGE queue" failures, and issue-order round-robin can
    # diverge from scheduled order.
    from concourse.tile_scheduler import PROC_NAMES
    for blk in nc.m.functions[0].blocks:
        for ins in blk.instructions:
            if ins.opcode == "DMAGatherAnt":
                proc = PROC_NAMES[ins.bass_scheduled_proc]
                assert proc.startswith("DMASW"), proc
                ins.queue_num = int(proc[5:]) % cfg.queues

    nc.compile()
    return nc


# ---------------------------------------------------------------- host driver


NPDT = {"f32": np.float32, "bf16": ml_dtypes.bfloat16, "f16": np.float16}


def make_in_maps(cfg: Cfg, ph1: PhasePlan, ph2: PhasePlan, v, w, mu):
    p1np, p2np = NPDT[cfg.p1dt], NPDT[cfg.p2dt]
    v_lo = np.ascontiguousarray(v[:cfg.half].astype(p1np))
    v_hi = np.ascontiguousarray(v[cfg.half:].astype(p1np))
    w_lo = np.ascontiguousarray(w[:cfg.half].astype(p2np))
    w_hi = np.ascontiguousarray(w[cfg.half:].astype(p2np))
    mu_col = np.broadcast_to(mu.reshape(-1)[:, None], (128, 1)).astype(np.float32)
    mu_col = np.ascontiguousarray(mu_col)
    in_maps = []
    for c in range(cfg.ncores):
        m = {
            "v_lo": v_lo, "v_hi": v_hi, "w_lo": w_lo, "w_hi": w_hi,
            "mu_col": mu_col,
            "idx1": ph1.idx_dram[c], "idx2": ph2.idx_dram[c],
            "wgt1": ph1.wgt_dram[c], "rl1": ph1.rl_dram[c],
            "rl2": ph2.rl_dram[c],
        }
        in_maps.append(m)
    return in_maps


def prepare(cfg: Cfg, ij, r, i, j):
    ph1 = plan_phase(cfg, ij[0], ij[1], r)
    ph2 = plan_phase(cfg, i, j, np.ones(len(i), np.float32))
    return ph1, ph2


_prog_cache = {}


def kernel(ij, r, m, i, j, v, mu, w, b, cfg: Cfg = FULL, _return_parts=False,
           _run_kwargs=None):
    from concourse.bass_utils import run_bass_kernel_spmd

    ij = np.asarray(ij)
    r = np.asarray(r, np.float32)
    i = np.asarray(i)
    j = np.asarray(j)
    v = np.asarray(v, np.float32)
    w = np.asarray(w, np.float32)
    mu = np.asarray(mu, np.float32)
    b = np.asarray(b, np.float32)
    assert int(m) == cfg.M

    ph1, ph2 = prepare(cfg, ij, r, i, j)
    key = (cfg.M, cfg.N, ph1.total_chunks, ph2.total_chunks,
           tuple(ph1.groups), tuple(ph2.groups))
    if key not in _prog_cache:
        _prog_cache.clear()
        _prog_cache[key] = build_program(cfg, ph1, ph2)
    nc = _prog_cache[key]

    in_maps = make_in_maps(cfg, ph1, ph2, v, w, mu)
    res = run_bass_kernel_spmd(nc, in_maps, list(range(cfg.ncores)),
                               **(_run_kwargs or {}))
    parts = [res.results[c]["s_out"] for c in range(cfg.ncores)]
    s = np.float32(sum(np.asarray(p, np.float64).sum() for p in parts))
    out = s + b[j]
    if _return_parts:
        return out, res
    return out



# revision 18
# speedup vs baseline: 1.1031x; 1.1031x over previous
"""AutoRec forward kernel for Trainium2, 8-core SPMD.

Math (see reference):
    agg = segment_sum(r[:,None] * v[cols], rows, m)     # sparse (m,n) @ v
    h   = sigmoid(agg + mu)                             # (M, D)
    s   = sum(h[i] * w[j])                              # global scalar over E pairs
    out = s + b[j]                                      # (E,)

Device strategy (per core, users sharded):
  Each core owns RPC = 6272 rows (users). Both heavy stages are instances of
  one primitive: "gather rows from a replicated table, weight them, and
  segment-sum into a local per-row accumulator":
    phase 1: table=v (bf16), weights=r,     rows=ij[0], cols=ij[1] -> aggT
    phase 2: table=w (f32),  weights=1.0,   rows=i,     cols=j     -> aT
          (sum_e h[i_e] * w[j_e] = sum_u h[u] . A[u],  A[u] = sum_{i_e=u} w[j_e])
  The segment-sum runs on the tensor engine: for each chunk of 128 edges the
  gathered rows form the stationary operand [128e, 128d]; a one-hot matrix
  P[e, wrow] = weight_e * (local_row_e == wrow) built on DVE is the moving
  operand; psum accumulates aggT[d, wrow] over a 64-row window. Edges are
  pre-sorted by (table-half, window) on the host so windows are contiguous,
  and the static schedule (max chunk count per group across cores) is shared
  by all cores so one SPMD program serves all 8.
  Tables are split in two 25000-row halves because dma_gather indices are
  int16. Finally h = sigmoid(aggT + mu) in one ACT op and
  s_part = sum(hT * aT) reduced on DVE; the host sums the 8 partials and
  broadcasts s + b[j] (a trivial O(E) numpy gather).
"""

import math
from dataclasses import dataclass, field

import ml_dtypes
import numpy as np

# ---------------------------------------------------------------- config

CHUNK = 128  # edges per matmul (contraction = partition dim)
IDX_WRAP = 16  # dma_gather index wrap


@dataclass
class Cfg:
    M: int = 50000          # users (rows of spmm)
    dma_scratch: int = 16384  # SWDGE descriptor carveout (bytes)
    N: int = 50000          # items (table rows)
    D: int = 128            # feature dim (must be 128)
    ncores: int = 8
    rpc: int = 6272         # rows per core (multiple of window)
    window: int = 64        # row-window of regular groups
    bank: int = 512         # psum-bank row-window of overflow groups
    half: int = 25000       # table split (int16 index limit)
    call_chunks: int = 8   # chunks per dma_gather call (ring = dma_scratch/16 descs/queue)
    p1dt: str = "f16"       # value dtype of phase-1 gathers / one-hot
    p2dt: str = "f16"       # value dtype of phase-2 gathers / one-hot
    ttb: int = 512          # block size of the final fused mul-reduce
    queues: int = 4         # SWDGE queues to round-robin gather calls over
    host_p: bool = False    # precompute one-hot P on host, stream via HWDGE

    @property
    def nwin(self):
        return self.rpc // self.window

    def __post_init__(self):
        assert self.rpc % self.window == 0
        assert self.rpc * self.ncores >= self.M
        assert self.N <= 2 * self.half and self.half <= 32767
        assert self.D == 128


FULL = Cfg()

# ---------------------------------------------------------------- host plan


@dataclass
class PhasePlan:
    groups: list          # [(kind, hf, tgt, n_chunks)]; kind 'w'=window 'b'=bank
    calls: list           # [(hf, overflow?, chunk_start, n_chunks)]
    total_chunks: int
    # per-core packed arrays
    idx_dram: list        # [ncores] int16 [128, total_chunks*8]
    wgt_dram: list        # [ncores] f16 [128, total_chunks] per-edge weights
    rl_dram: list         # [ncores] f16 [128, total_chunks] per-edge local row (-1 pad)


def _wrap_idxs(ii: np.ndarray) -> np.ndarray:
    """[n] -> [128, n/16] wrapped (t -> (t%16, t//16)), replicated x8."""
    n = len(ii)
    a = ii.reshape(n // IDX_WRAP, IDX_WRAP).T
    return np.tile(a, (8, 1))


def _run_rank(key: np.ndarray) -> np.ndarray:
    """rank within each equal-key run of a sorted key array."""
    n = len(key)
    if n == 0:
        return np.zeros(0, np.int64)
    first = np.r_[True, key[1:] != key[:-1]]
    run_start = np.maximum.accumulate(np.where(first, np.arange(n), 0))
    return np.arange(n) - run_start


def plan_phase(cfg: Cfg, rows, cols, wgts) -> PhasePlan:
    """Fixed chunk count per (hf, 64-row window) group (from the cross-core
    mean) + per (hf, 512-row psum bank) overflow groups sized by the
    cross-core max — all cores share one static schedule with ~2% padding."""
    rows = np.asarray(rows, np.int64)
    cols = np.asarray(cols, np.int64)
    wgts = np.asarray(wgts, np.float32)
    ncores, Wd, BK = cfg.ncores, cfg.window, cfg.bank
    nwin = cfg.rpc // Wd
    nbank = -(-cfg.rpc // BK)
    wpb = BK // Wd

    core = rows // cfg.rpc
    local = rows - core * cfg.rpc
    win = local // Wd
    hf = (cols >= cfg.half).astype(np.int64)
    idx16 = (cols - hf * cfg.half).astype(np.int16)

    key = (core * 2 + hf) * nwin + win
    counts = np.bincount(key, minlength=ncores * 2 * nwin).reshape(
        ncores, 2, nwin)
    kg = np.maximum(1, np.rint(counts.mean(axis=0) / CHUNK).astype(np.int64))
    cap = kg * CHUNK                               # [2, nwin]
    winbank = np.arange(nwin) // wpb
    ov = np.maximum(counts - cap[None], 0)         # [ncores, 2, nwin]
    ovb = np.zeros((ncores, 2, nbank), np.int64)
    for c in range(ncores):
        for h in range(2):
            ovb[c, h] = np.bincount(winbank, weights=ov[c, h],
                                    minlength=nbank).astype(np.int64)
    kov = -(-ovb.max(axis=0) // CHUNK)             # [2, nbank]

    groups = []
    gbaseW = np.zeros((2, nwin), np.int64)
    gbaseB = np.zeros((2, nbank), np.int64)
    acc = 0
    segs = []  # [(hf, of, c0, c1)]
    for h in range(2):
        c0 = acc
        for w in range(nwin):
            groups.append(("w", h, w, int(kg[h, w])))
            gbaseW[h, w] = acc
            acc += int(kg[h, w])
        segs.append((h, 0, c0, acc))
        c0 = acc
        for b in range(nbank):
            n = int(kov[h, b])
            if n == 0:
                continue
            groups.append(("b", h, b, n))
            gbaseB[h, b] = acc
            acc += n
        segs.append((h, 1, c0, acc))
    total_chunks = acc

    calls = []
    for (h, of, c0, c1) in segs:
        off = c0
        while off < c1:
            n = min(cfg.call_chunks, c1 - off)
            calls.append((h, of, off, n))
            off += n

    idx_l, wgt_l, rl_l = [], [], []
    for c in range(ncores):
        mask = core == c
        eh, ew, el = hf[mask], win[mask], local[mask]
        ei, ewgt = idx16[mask], wgts[mask]
        order = np.lexsort((ew, eh))
        eh, ew, el, ei, ewgt = (a[order] for a in (eh, ew, el, ei, ewgt))
        rank = _run_rank(eh * nwin + ew)
        reg = rank < cap[eh, ew]

        pos = np.empty(len(rank), np.int64)
        rlv = np.empty(len(rank), np.float64)
        pos[reg] = gbaseW[eh[reg], ew[reg]] * CHUNK + rank[reg]
        rlv[reg] = el[reg] - ew[reg] * Wd
        # overflow edges: rank within (hf, bank), stream order already
        # (hf, win)-sorted so a stable pass per (hf, bank) suffices
        oe = ~reg
        ob = ew[oe] // wpb
        orank = _run_rank(eh[oe] * nbank + ob)  # runs are contiguous: sorted
        assert np.all(orank < kov[eh[oe], ob] * CHUNK)
        pos[oe] = gbaseB[eh[oe], ob] * CHUNK + orank
        rlv[oe] = el[oe] - ob * BK

        idx_full = np.zeros(total_chunks * CHUNK, np.int16)
        wgt_full = np.zeros(total_chunks * CHUNK, np.float32)
        # pad slots: rl=-1 so the device one-hot (iota==rl) is all-zero
        rl_full = np.full(total_chunks * CHUNK, -1.0, np.float32)
        idx_full[pos] = ei
        wgt_full[pos] = ewgt
        rl_full[pos] = rlv

        # wrap idx per call
        parts = []
        for (_h, _of, c0, n) in calls:
            parts.append(_wrap_idxs(idx_full[c0 * CHUNK:(c0 + n) * CHUNK]))
        idx_l.append(np.concatenate(parts, axis=1))
        wgt_l.append(wgt_full.reshape(-1, CHUNK).T.astype(np.float16))
        rl_l.append(rl_full.reshape(-1, CHUNK).T.astype(np.float16))

    return PhasePlan(groups, calls, total_chunks, idx_l, wgt_l, rl_l)


# ---------------------------------------------------------------- device build


def build_program(cfg: Cfg, ph1: PhasePlan, ph2: PhasePlan):
    import concourse.bacc as bacc
    import concourse.bass as bass
    import concourse.mybir as mybir
    import concourse.tile as tile

    f32 = mybir.dt.float32
    f16 = mybir.dt.float16
    i16 = mybir.dt.int16
    DTMAP = {"f32": f32, "bf16": mybir.dt.bfloat16, "f16": mybir.dt.float16}
    p1dt, p2dt = DTMAP[cfg.p1dt], DTMAP[cfg.p2dt]
    P, Wd, RPC = 128, cfg.window, cfg.rpc
    n_hi = cfg.N - cfg.half

    nc = bacc.Bacc("TRN2", target_bir_lowering=False, debug=False,
                   dynamic_dma_scratch_size=cfg.dma_scratch,
                   num_swdge_queues=cfg.queues)

    v_lo = nc.dram_tensor("v_lo", [cfg.half, cfg.D], p1dt, kind="ExternalInput")
    v_hi = nc.dram_tensor("v_hi", [n_hi, cfg.D], p1dt, kind="ExternalInput")
    w_lo = nc.dram_tensor("w_lo", [cfg.half, cfg.D], p2dt, kind="ExternalInput")
    w_hi = nc.dram_tensor("w_hi", [n_hi, cfg.D], p2dt, kind="ExternalInput")
    mu_c = nc.dram_tensor("mu_col", [P, 1], f32, kind="ExternalInput")

    idx1 = nc.dram_tensor("idx1", [P, ph1.total_chunks * 8], i16,
                          kind="ExternalInput")
    idx2 = nc.dram_tensor("idx2", [P, ph2.total_chunks * 8], i16,
                          kind="ExternalInput")
    wgt1 = nc.dram_tensor("wgt1", [P, ph1.total_chunks], f16,
                          kind="ExternalInput")
    rl1 = nc.dram_tensor("rl1", [P, ph1.total_chunks], f16,
                         kind="ExternalInput")
    rl2 = nc.dram_tensor("rl2", [P, ph2.total_chunks], f16,
                         kind="ExternalInput")
    s_out = nc.dram_tensor("s_out", [P, 1], f32, kind="ExternalOutput")

    BK = cfg.bank
    with tile.TileContext(nc) as tc:
        with (
            tc.tile_pool(name="const", bufs=1) as cpool,
            tc.tile_pool(name="idxp", bufs=8) as ipool,
            tc.tile_pool(name="g1", bufs=6) as g1pool,
            tc.tile_pool(name="g2", bufs=6) as g2pool,
            tc.tile_pool(name="pp", bufs=6) as ppool,
            tc.tile_pool(name="po", bufs=2) as popool,
            tc.tile_pool(name="ev", bufs=4) as evpool,
            tc.tile_pool(name="psw", bufs=4, space="PSUM") as pswin,
            tc.tile_pool(name="psb", bufs=2, space="PSUM") as psbank,
        ):
            # constants
            mu_t = cpool.tile([P, 1], f32, tag="mu")
            nc.sync.dma_start(mu_t[:], mu_c[:])
            iota_t = cpool.tile([P, BK], f16, tag="iota")
            nc.gpsimd.iota(iota_t[:], pattern=[[1, BK]], base=0,
                           channel_multiplier=0,
                           allow_small_or_imprecise_dtypes=True)

            acc1 = cpool.tile([P, RPC], f32, tag="acc1")
            acc2 = cpool.tile([P, RPC], f32, tag="acc2")
            nc.vector.memset(acc1[:], 0.0)
            nc.vector.memset(acc2[:], 0.0)

            wg1_t = cpool.tile([P, ph1.total_chunks], f16, tag="wg1")
            rl1_t = cpool.tile([P, ph1.total_chunks], f16, tag="rl1")
            rl2_t = cpool.tile([P, ph2.total_chunks], f16, tag="rl2")
            nc.sync.dma_start(wg1_t[:], wgt1[:])
            nc.sync.dma_start(rl1_t[:], rl1[:])
            nc.sync.dma_start(rl2_t[:], rl2[:])

            def run_phase(pl: PhasePlan, tabs, idx_dram, wg_t, rl_t,
                          acc, gpool, pdt):
                # chunk id -> (kind, tgt, first?, last?)
                chunk_group = {}
                base = 0
                for (kind, h, tgt, n) in pl.groups:
                    for k in range(n):
                        chunk_group[base + k] = (kind, tgt, k == 0, k == n - 1)
                    base += n
                open_ps = None
                for (h, of, c0, n) in pl.calls:
                    it = ipool.tile([P, cfg.call_chunks * 8], i16, tag="idx")
                    nc.sync.dma_start(it[:, : n * 8],
                                      idx_dram[:, c0 * 8:(c0 + n) * 8])
                    gt = gpool.tile([P, cfg.call_chunks, cfg.D], pdt, tag="g")
                    nidx = n * CHUNK
                    nc.gpsimd.dma_gather(
                        gt[:, :n, :], tabs[h][:], it[:, : n * 8],
                        num_idxs=nidx, num_idxs_reg=nidx, elem_size=cfg.D,
                        queue_num=0,
                    )
                    # one-hot for the whole call in two (or one) DVE ops:
                    # P[p, k, x] = wgt[p, c0+k] * (iota[x] == rl[p, c0+k])
                    W = BK if of else Wd
                    pool = popool if of else ppool
                    p_t = pool.tile([P, cfg.call_chunks, W], pdt,
                                    tag="po" if of else "p")
                    nc.vector.tensor_tensor(
                        out=p_t[:, :n, :],
                        in0=iota_t[:, None, :W].broadcast_to([P, n, W]),
                        in1=rl_t[:, c0:c0 + n, None].broadcast_to([P, n, W]),
                        op=mybir.AluOpType.is_equal)
                    if wg_t is not None:
                        nc.vector.tensor_tensor(
                            out=p_t[:, :n, :], in0=p_t[:, :n, :],
                            in1=wg_t[:, c0:c0 + n, None].broadcast_to(
                                [P, n, W]),
                            op=mybir.AluOpType.mult)
                    for k in range(n):
                        cid = c0 + k
                        kind, tgt, first, last = chunk_group[cid]
                        if kind == "w":
                            off, width = tgt * Wd, Wd
                        else:
                            off = tgt * BK
                            width = min(BK, RPC - off)
                        if first:
                            open_ps = (psbank if kind == "b" else pswin).tile(
                                [P, width], mybir.dt.float32,
                                tag="psb" if kind == "b" else "ps")
                        nc.tensor.matmul(open_ps[:], gt[:, k, :],
                                         p_t[:, k, :width],
                                         start=first, stop=last)
                        if last:
                            sl = acc[:, off:off + width]
                            nc.vector.tensor_tensor(
                                out=sl, in0=sl, in1=open_ps[:],
                                op=mybir.AluOpType.add)

            run_phase(ph1, (v_lo, v_hi), idx1, wg1_t, rl1_t, acc1,
                      g1pool, p1dt)
            run_phase(ph2, (w_lo, w_hi), idx2, None, rl2_t, acc2,
                      g2pool, p2dt)

            # h = sigmoid(aggT + mu)  (in place on acc1)
            nc.scalar.activation(acc1[:], acc1[:],
                                 mybir.ActivationFunctionType.Sigmoid,
                                 bias=mu_t[:, 0:1], scale=1.0)

            # s_part[p] = sum_d sum_u h[p,u]*A[p,u]  blockwise fused mul+reduce
            nblk = math.ceil(RPC / cfg.ttb)
            s_cols = cpool.tile([P, nblk], f32, tag="scols")
            for b in range(nblk):
                lo = b * cfg.ttb
                hi = min(RPC, lo + cfg.ttb)
                tmp = evpool.tile([P, cfg.ttb], f32, tag="tmp")
                nc.vector.tensor_tensor(
                    out=tmp[:, : hi - lo],
                    in0=acc1[:, lo:hi], in1=acc2[:, lo:hi],
                    op=mybir.AluOpType.mult)
                nc.vector.tensor_reduce(
                    s_cols[:, b:b + 1], tmp[:, : hi - lo],
                    axis=mybir.AxisListType.X, op=mybir.AluOpType.add)
            s_t = cpool.tile([P, 1], f32, tag="sfin")
            nc.vector.tensor_reduce(s_t[:], s_cols[:], axis=mybir.AxisListType.X,
                                    op=mybir.AluOpType.add)
            nc.sync.dma_start(s_out[:], s_t[:])

    # Align SWDGE queue assignment with Tile's DMASW lane assignment (which
    # follows scheduled order): queue = lane % queues. A mismatch trips
    # "sem locked to SWDGE queue" failures, and issue-order round-robin can
    # diverge from scheduled order.
    from concourse.tile_scheduler import PROC_NAMES
    for blk in nc.m.functions[0].blocks:
        for ins in blk.instructions:
            if ins.opcode == "DMAGatherAnt":
                proc = PROC_NAMES[ins.bass_scheduled_proc]
                assert proc.startswith("DMASW"), proc
                ins.queue_num = int(proc[5:]) % cfg.queues

    nc.compile()
    return nc


# ---------------------------------------------------------------- host driver


NPDT = {"f32": np.float32, "bf16": ml_dtypes.bfloat16, "f16": np.float16}


def make_in_maps(cfg: Cfg, ph1: PhasePlan, ph2: PhasePlan, v, w, mu):
    p1np, p2np = NPDT[cfg.p1dt], NPDT[cfg.p2dt]
    v_lo = np.ascontiguousarray(v[:cfg.half].astype(p1np))
    v_hi = np.ascontiguousarray(v[cfg.half:].astype(p1np))
    w_lo = np.ascontiguousarray(w[:cfg.half].astype(p2np))
    w_hi = np.ascontiguousarray(w[cfg.half:].astype(p2np))
    mu_col = np.broadcast_to(mu.reshape(-1)[:, None], (128, 1)).astype(np.float32)
    mu_col = np.ascontiguousarray(mu_col)
    in_maps = []
    for c in range(cfg.ncores):
        m = {
            "v_lo": v_lo, "v_hi": v_hi, "w_lo": w_lo, "w_hi": w_hi,
            "mu_col": mu_col,
            "idx1": ph1.idx_dram[c], "idx2": ph2.idx_dram[c],
            "wgt1": ph1.wgt_dram[c], "rl1": ph1.rl_dram[c],
            "rl2": ph2.rl_dram[c],
        }
        in_maps.append(m)
    return in_maps


def prepare(cfg: Cfg, ij, r, i, j):
    ph1 = plan_phase(cfg, ij[0], ij[1], r)
    ph2 = plan_phase(cfg, i, j, np.ones(len(i), np.float32))
    return ph1, ph2


_prog_cache = {}


def kernel(ij, r, m, i, j, v, mu, w, b, cfg: Cfg = FULL, _return_parts=False,
           _run_kwargs=None):
    from concourse.bass_utils import run_bass_kernel_spmd

    ij = np.asarray(ij)
    r = np.asarray(r, np.float32)
    i = np.asarray(i)
    j = np.asarray(j)
    v = np.asarray(v, np.float32)
    w = np.asarray(w, np.float32)
    mu = np.asarray(mu, np.float32)
    b = np.asarray(b, np.float32)
    assert int(m) == cfg.M

    ph1, ph2 = prepare(cfg, ij, r, i, j)
    key = (cfg.M, cfg.N, ph1.total_chunks, ph2.total_chunks,
           tuple(ph1.groups), tuple(ph2.groups))
    if key not in _prog_cache:
        _prog_cache.clear()
        _prog_cache[key] = build_program(cfg, ph1, ph2)
    nc = _prog_cache[key]

    in_maps = make_in_maps(cfg, ph1, ph2, v, w, mu)
    res = run_bass_kernel_spmd(nc, in_maps, list(range(cfg.ncores)),
                               **(_run_kwargs or {}))
    parts = [res.results[c]["s_out"] for c in range(cfg.ncores)]
    s = np.float32(sum(np.asarray(p, np.float64).sum() for p in parts))
    out = s + b[j]
    if _return_parts:
        return out, res
    return out

